# revision 14
# baseline (speedup 1.0000x reference)
"""Trainium2 Bass kernel for nn_GNN_Model (gnn_message_passing).

Data-parallel over B=16384 (query,mv) pairs across 8 cores (2048 each).

Gather architecture (the int16-index dma_gather is the only fast gather
on this image, so indices are bucket-decomposed):
  - feats table fp16 in DRAM; viewed as 16 buckets of 31250 rows so
    bucket-local indices fit int16.
  - per super-group (~24K rows): 16 bucket dma_gathers (one SWDGE
    instruction each, ~0.34ns/row) land rows bucket-sorted in SBUF,
    then stream to a DRAM scratch; a second dma_gather with host-built
    int16 permutation indices reads rows back in batch order.
  - PE transposes (fp16) make xt feature-major; gates/products as usual.

Compute per 2048-row group: stationary-weight gate matmuls (f,i) ->
ACT sigmoid(+bias), ACT tanh -> DVE products + segmented reduce
(mean over 32 neighbors). Tail: Wo/Wmv/W1/W2 feature-major.
"""

import os

import numpy as np

import concourse.bass as bass
import concourse.mybir as mybir
import concourse.tile as tile
from concourse import bacc
from concourse.bass_utils import run_bass_kernel_spmd
from concourse.masks import make_identity

N = 500000
D = 128
H = 256
B = 16384
KN = 32
NCORES = 8
BC = B // NCORES          # 2048 rows per core
NROW = BC * KN + 2 * BC   # 69632 gathered rows per core
NBKT = 16
BKT = N // NBKT           # 31250 rows per bucket (int16-addressable)
GROUP = 2048              # rows per compute group
NG = NROW // GROUP        # 34 groups (32 nbr + q + mv)
SG_GROUPS = (12, 12, 10)  # compute groups per super-group
FP16 = mybir.dt.float16
F32 = mybir.dt.float32
I16 = mybir.dt.int16
LAST_EXEC_NS = None


def _build(b2_imm: float, sg_meta):
    """sg_meta: list of dicts with keys:
    t (padded row count), starts[16], counts[16] (padded per-bucket)."""
    nc = bacc.Bacc(None, target_bir_lowering=False)

    feats = nc.dram_tensor("feats", [N, D], FP16, kind="ExternalInput")
    t_all = sum(m["t"] for m in sg_meta)
    idx16 = nc.dram_tensor("idx16", [128, t_all // 16], I16, kind="ExternalInput")
    perm16 = nc.dram_tensor("perm16", [128, NG * (GROUP // 16)], I16,
                            kind="ExternalInput")
    w_names = ["wf", "wi", "wo", "wmva", "wmvb", "w1qa", "w1qb", "w1ma", "w1mb"]
    wt = {n: nc.dram_tensor(n, [128, 128], FP16, kind="ExternalInput") for n in w_names}
    wt["w2a"] = nc.dram_tensor("w2a", [128, 1], FP16, kind="ExternalInput")
    wt["w2b"] = nc.dram_tensor("w2b", [128, 1], FP16, kind="ExternalInput")
    b_names = ["bf", "bi", "bo", "b1a", "b1b"]
    bt = {n: nc.dram_tensor(n, [128, 1], F32, kind="ExternalInput") for n in b_names}
    out = nc.dram_tensor("out", [1, BC], F32, kind="ExternalOutput")
    scratch = [nc.dram_tensor(f"scr{i}", [m["t"], D], FP16, kind="Internal")
               for i, m in enumerate(sg_meta)]

    SIG = mybir.ActivationFunctionType.Sigmoid
    TANH = mybir.ActivationFunctionType.Tanh
    COPY = mybir.ActivationFunctionType.Copy
    MUL = mybir.AluOpType.mult
    ADD = mybir.AluOpType.add
    MAXOP = mybir.AluOpType.max

    t_max = max(m["t"] for m in sg_meta)

    with tile.TileContext(nc) as tc:
        with (
            tc.tile_pool(name="const", bufs=1) as cp,
            tc.tile_pool(name="stage", bufs=1) as stp,
            tc.tile_pool(name="xtr", bufs=2) as xrp,
            tc.tile_pool(name="xt", bufs=2) as xtp,
            tc.tile_pool(name="gate", bufs=2) as gp,
            tc.tile_pool(name="ve", bufs=2) as vp,
        ):
            ident = cp.tile([128, 128], FP16)
            make_identity(nc, ident[:])
            idx_t = cp.tile([128, t_all // 16], I16)
            nc.sync.dma_start(out=idx_t[:], in_=idx16[:])
            perm_t = cp.tile([128, NG * (GROUP // 16)], I16)
            nc.sync.dma_start(out=perm_t[:], in_=perm16[:])
            w = {}
            for n, dr in wt.items():
                w[n] = cp.tile([128, dr.shape[1]], FP16, tag=f"w_{n}", name=f"w_{n}")
                nc.sync.dma_start(out=w[n][:], in_=dr[:])
            bias = {}
            for n, dr in bt.items():
                bias[n] = cp.tile([128, 1], F32, tag=f"b_{n}", name=f"b_{n}")
                nc.sync.dma_start(out=bias[n][:], in_=dr[:])
            c16 = cp.tile([128, BC], FP16)   # c.T (unscaled sum over k)
            qt_sb = cp.tile([128, BC], FP16)
            mvt_sb = cp.tile([128, BC], FP16)

            def transpose_group(xtr, xt):
                # xtr row-major stripe [128, 16*128] -> xt feature-major
                for hh in range(2):
                    xt_ps = xpp.tile([128, 1024], FP16, tag="xtps")
                    for t4 in range(8):
                        ch = hh * 8 + t4
                        nc.tensor.transpose(
                            xt_ps[:, t4 * 128:(t4 + 1) * 128],
                            xtr[:, ch * 128:(ch + 1) * 128], ident[:])
                    nc.vector.tensor_copy(
                        out=xt[:, hh * 1024:(hh + 1) * 1024], in_=xt_ps[:])

            def compute_group(xt, g):
                # gates + products + segmented reduce for one 2048-col group
                f_sb = gp.tile([128, GROUP], FP16, tag="f")
                i_sb = gp.tile([128, GROUP], FP16, tag="i")
                t_sb = gp.tile([128, GROUP], FP16, tag="t")
                for hh in range(2):
                    cols = slice(hh * 1024, (hh + 1) * 1024)
                    f_ps = gpp.tile([128, 1024], F32, tag="fps")
                    for s in range(2):
                        c0 = hh * 1024 + s * 512
                        nc.tensor.matmul(f_ps[:, s * 512:(s + 1) * 512],
                                         lhsT=w["wf"][:], rhs=xt[:, c0:c0 + 512],
                                         start=True, stop=True)
                    nc.scalar.activation(f_sb[:, cols], f_ps[:], SIG,
                                         bias=bias["bf"][:])
                    i_ps = gpp.tile([128, 1024], F32, tag="ips")
                    for s in range(2):
                        c0 = hh * 1024 + s * 512
                        nc.tensor.matmul(i_ps[:, s * 512:(s + 1) * 512],
                                         lhsT=w["wi"][:], rhs=xt[:, c0:c0 + 512],
                                         start=True, stop=True)
                    nc.scalar.activation(i_sb[:, cols], i_ps[:], SIG,
                                         bias=bias["bi"][:])
                nc.scalar.activation(t_sb[:], xt[:], TANH)
                fi = vp.tile([128, GROUP], FP16, tag="fi")
                prod = vp.tile([128, GROUP], FP16, tag="prod")
                nc.vector.tensor_tensor(out=fi[:], in0=f_sb[:], in1=i_sb[:], op=MUL)
                nc.vector.tensor_tensor(out=prod[:], in0=fi[:], in1=t_sb[:], op=MUL)
                with nc.allow_low_precision(reason="32-term mean, fp16 ok"):
                    nc.vector.tensor_reduce(
                        out=c16[:, g * (GROUP // KN):(g + 1) * (GROUP // KN)],
                        in_=prod[:].rearrange("p (b k) -> p b k", k=KN),
                        axis=mybir.AxisListType.X,
                        op=ADD,
                    )

            with (
                tc.tile_pool(name="xps", bufs=2, space="PSUM") as xpp,
                tc.tile_pool(name="gpsum", bufs=1, space="PSUM") as gpp,
            ):
                g_global = 0
                idx_off = 0
                for sgi, meta in enumerate(sg_meta):
                    t_sg = meta["t"]
                    lblk = t_sg // 128
                    stage = stp.tile([128, t_max], FP16, tag="stage")
                    scr = scratch[sgi]
                    # bucket gathers + scratch writeback (<=1024 idx per
                    # instruction: SWDGE descriptor ring holds 1024)
                    for b in range(NBKT):
                        s0, cnt = meta["starts"][b], meta["counts"][b]
                        if cnt == 0:
                            continue
                        for c0 in range(0, cnt, 1024):
                            n = min(1024, cnt - c0)
                            a = s0 + c0
                            nc.gpsimd.dma_gather(
                                stage[:, a:a + n].rearrange(
                                    "p (m f) -> p m f", f=128),
                                feats[b * BKT:(b + 1) * BKT],
                                idx_t[:, (idx_off + a) // 16:(idx_off + a + n) // 16],
                                n, n, D,
                            )
                        # stage slot (p, blk) -> scratch row p*lblk + blk
                        nc.sync.dma_start(
                            out=scr[:].rearrange(
                                "(p blk) f -> p (blk f)", p=128
                            )[:, s0:s0 + cnt],
                            in_=stage[:, s0:s0 + cnt],
                        )
                    idx_off += t_sg
                    # permuted re-gather per compute group + compute
                    for gl in range(SG_GROUPS[sgi]):
                        g = g_global
                        xtr = xrp.tile([128, GROUP], FP16, tag="xtr")
                        for c0 in range(0, GROUP, 1024):
                            nc.gpsimd.dma_gather(
                                xtr[:, c0:c0 + 1024].rearrange(
                                    "p (m f) -> p m f", f=128),
                                scr[:],
                                perm_t[:, (g * GROUP + c0) // 16:
                                       (g * GROUP + c0 + 1024) // 16],
                                1024, 1024, D,
                            )
                        if g < NG - 2:
                            xt = xtp.tile([128, GROUP], FP16, tag="xt")
                            transpose_group(xtr, xt)
                            compute_group(xt, g)
                        elif g == NG - 2:
                            transpose_group(xtr, qt_sb)
                        else:
                            transpose_group(xtr, mvt_sb)
                        g_global += 1

                # ---- tail ----
                tc_sb = cp.tile([128, BC], FP16)
                nc.scalar.activation(tc_sb[:], c16[:], TANH, scale=1.0 / KN)
                emb = {}
                for sname, src_t in (("q", qt_sb), ("mv", mvt_sb)):
                    o_sb = vp.tile([128, BC], FP16, tag="fi")
                    for hh in range(2):
                        cols = slice(hh * 1024, (hh + 1) * 1024)
                        o_ps = gpp.tile([128, 1024], F32, tag="fps")
                        for s in range(2):
                            sl = slice(hh * 1024 + s * 512, hh * 1024 + (s + 1) * 512)
                            nc.tensor.matmul(o_ps[:, s * 512:(s + 1) * 512],
                                             lhsT=w["wo"][:], rhs=src_t[:, sl],
                                             start=True, stop=True)
                        nc.scalar.activation(o_sb[:, cols], o_ps[:], SIG,
                                             bias=bias["bo"][:])
                    h_sb = vp.tile([128, BC], FP16, tag="prod")
                    nc.vector.tensor_tensor(out=h_sb[:], in0=o_sb[:],
                                            in1=tc_sb[:], op=MUL)
                    emb_sb = cp.tile([128, BC], FP16, tag=f"emb_{sname}",
                                     name=f"emb_{sname}")
                    for hh in range(2):
                        cols = slice(hh * 1024, (hh + 1) * 1024)
                        e_ps = gpp.tile([128, 1024], F32, tag="ips")
                        for s in range(2):
                            sl = slice(hh * 1024 + s * 512, hh * 1024 + (s + 1) * 512)
                            psl = slice(s * 512, (s + 1) * 512)
                            nc.tensor.matmul(e_ps[:, psl], lhsT=w["wmva"][:],
                                             rhs=src_t[:, sl], start=True, stop=False)
                            nc.tensor.matmul(e_ps[:, psl], lhsT=w["wmvb"][:],
                                             rhs=h_sb[:, sl], start=False, stop=True)
                        nc.vector.tensor_copy(out=emb_sb[:, cols], in_=e_ps[:])
                    emb[sname] = emb_sb

                hid = [cp.tile([128, BC], FP16, tag=f"hid{h}", name=f"hid{h}")
                       for h in range(2)]
                for h in range(2):
                    wq = w["w1qa"] if h == 0 else w["w1qb"]
                    wm = w["w1ma"] if h == 0 else w["w1mb"]
                    b1 = bias["b1a"] if h == 0 else bias["b1b"]
                    for hh in range(2):
                        cols = slice(hh * 1024, (hh + 1) * 1024)
                        h_ps = gpp.tile([128, 1024], F32, tag="fps")
                        for s in range(2):
                            sl = slice(hh * 1024 + s * 512, hh * 1024 + (s + 1) * 512)
                            psl = slice(s * 512, (s + 1) * 512)
                            nc.tensor.matmul(h_ps[:, psl], lhsT=wq[:],
                                             rhs=emb["q"][:, sl], start=True, stop=False)
                            nc.tensor.matmul(h_ps[:, psl], lhsT=wm[:],
                                             rhs=emb["mv"][:, sl], start=False, stop=True)
                        nc.vector.tensor_scalar(
                            out=hid[h][:, cols], in0=h_ps[:], scalar1=b1[:],
                            scalar2=0.0, op0=ADD, op1=MAXOP,
                        )

                ben_sb = cp.tile([1, BC], F32)
                for hh in range(2):
                    cols = slice(hh * 1024, (hh + 1) * 1024)
                    b_ps = gpp.tile([1, 1024], F32, tag="bps")
                    for s in range(2):
                        sl = slice(hh * 1024 + s * 512, hh * 1024 + (s + 1) * 512)
                        psl = slice(s * 512, (s + 1) * 512)
                        nc.tensor.matmul(b_ps[:, psl], lhsT=w["w2a"][:],
                                         rhs=hid[0][:, sl], start=True, stop=False)
                        nc.tensor.matmul(b_ps[:, psl], lhsT=w["w2b"][:],
                                         rhs=hid[1][:, sl], start=False, stop=True)
                    nc.scalar.activation(ben_sb[:, cols], b_ps[:], COPY,
                                         bias=float(b2_imm))
            nc.sync.dma_start(out=out[:], in_=ben_sb[:])

    nc.compile()
    return nc


def _wrap16(vals):
    # flat [n] -> [128, n//16] with value i at [16k + i%16, i//16] for all k
    n = len(vals)
    outw = np.zeros((128, n // 16), dtype=np.int16)
    v = np.asarray(vals, dtype=np.int16).reshape(-1, 16).T  # [16, n//16]
    for k in range(8):
        outw[16 * k:16 * (k + 1), :] = v
    return outw


def _prep_core(all_rows):
    """Bucket-decompose one core's row list.

    Returns (idx16 [128, T/16], perm16 [128, NG*GROUP/16], sg_meta)."""
    sg_bounds = []
    a = 0
    for ngr in SG_GROUPS:
        sg_bounds.append((a, a + ngr * GROUP))
        a += ngr * GROUP
    idx_cols = []
    perm_cols = []
    sg_meta = []
    for (r0, r1) in sg_bounds:
        rows = all_rows[r0:r1]
        bkt = rows // BKT
        loc = (rows % BKT).astype(np.int16)
        order = np.argsort(bkt, kind="stable")
        counts = np.bincount(bkt, minlength=NBKT)
        pads = (-counts) % 128
        pcounts = counts + pads
        starts = np.zeros(NBKT, dtype=np.int64)
        starts[1:] = np.cumsum(pcounts)[:-1]
        t_sg = int(pcounts.sum())
        lblk = t_sg // 128
        # positions in padded bucket-sorted order
        pos = np.empty(len(rows), dtype=np.int64)
        csort = np.cumsum(counts)
        ranks = np.empty(len(rows), dtype=np.int64)
        ranks[order] = np.arange(len(rows))  # rank within global stable sort
        # rank within bucket = global sorted rank - cumulative count before bucket
        before = np.concatenate([[0], csort[:-1]])
        pos = starts[bkt] + (ranks - before[bkt])
        # idx16 for this sg
        idxv = np.zeros(t_sg, dtype=np.int16)
        srt = order
        bsrt = bkt[srt]
        within = np.arange(len(rows)) - before[bsrt]
        idxv[starts[bsrt] + within] = loc[srt]
        idx_cols.append(_wrap16(idxv))
        # perm: original row j -> scratch slot (pos% 128)*lblk + pos//128
        slots = (pos % 128) * lblk + pos // 128
        assert slots.max() < 32768
        for g in range(len(rows) // GROUP):
            perm_cols.append(_wrap16(slots[g * GROUP:(g + 1) * GROUP].astype(np.int16)))
        sg_meta.append({"t": t_sg, "starts": [int(x) for x in starts],
                        "counts": [int(x) for x in pcounts]})
    return (np.concatenate(idx_cols, axis=1),
            np.concatenate(perm_cols, axis=1), sg_meta)


def kernel(feats, query_idx, mv_idx, neighbor_idx,
           Wf, bf, Wi, bi, Wo, bo, Wmv, bmv, W1, b1, W2, b2):
    feats16 = np.ascontiguousarray(np.asarray(feats, dtype=np.float16))
    query_idx = np.asarray(query_idx).astype(np.int64)
    mv_idx = np.asarray(mv_idx).astype(np.int64)
    neighbor_idx = np.asarray(neighbor_idx).astype(np.int64)
    Wf, Wi, Wo = [np.asarray(x, np.float32) for x in (Wf, Wi, Wo)]
    Wmv, W1, W2 = [np.asarray(x, np.float32) for x in (Wmv, W1, W2)]
    bf, bi, bo, bmv, b1, b2 = [np.asarray(x, np.float32) for x in (bf, bi, bo, bmv, b1, b2)]

    b1_eff = b1 + W1.T @ np.concatenate([bmv, bmv])
    f16 = np.float16
    weights = {
        "wf": Wf.astype(f16), "wi": Wi.astype(f16), "wo": Wo.astype(f16),
        "wmva": Wmv[0:128].astype(f16), "wmvb": Wmv[128:256].astype(f16),
        "w1qa": W1[0:128, 0:128].astype(f16), "w1qb": W1[0:128, 128:256].astype(f16),
        "w1ma": W1[128:256, 0:128].astype(f16), "w1mb": W1[128:256, 128:256].astype(f16),
        "w2a": np.ascontiguousarray(W2[0:128]).astype(f16),
        "w2b": np.ascontiguousarray(W2[128:256]).astype(f16),
    }
    biases = {
        "bf": bf.reshape(128, 1), "bi": bi.reshape(128, 1), "bo": bo.reshape(128, 1),
        "b1a": b1_eff[0:128].reshape(128, 1).astype(np.float32),
        "b1b": b1_eff[128:256].reshape(128, 1).astype(np.float32),
    }

    in_maps = []
    sg_meta0 = None
    for c in range(NCORES):
        b0 = c * BC
        all_rows = np.concatenate([
            neighbor_idx[b0:b0 + BC].reshape(-1),
            query_idx[b0:b0 + BC],
            mv_idx[b0:b0 + BC],
        ])
        idx16, perm16, sg_meta = _prep_core(all_rows)
        if sg_meta0 is None:
            sg_meta0 = sg_meta
        else:
            # pad T to match core 0's compiled sizes: recompile-free requires
            # identical shapes; pad buckets so t matches by construction
            pass
        im = {"feats": feats16, "idx16": idx16, "perm16": perm16}
        im.update(weights)
        im.update({k: np.ascontiguousarray(v) for k, v in biases.items()})
        in_maps.append((im, sg_meta))

    # sg buckets differ per core; compile per unique meta signature
    # To keep one compile: pad every core's buckets to the max count per
    # (sg, bucket) across cores.
    max_counts = []
    for sgi in range(len(SG_GROUPS)):
        mc = [max(in_maps[c][1][sgi]["counts"][b] for c in range(NCORES))
              for b in range(NBKT)]
        max_counts.append(mc)
    # rebuild idx/perm with padded counts
    in_maps2 = []
    sg_meta_u = None
    for c in range(NCORES):
        b0 = c * BC
        all_rows = np.concatenate([
            neighbor_idx[b0:b0 + BC].reshape(-1),
            query_idx[b0:b0 + BC],
            mv_idx[b0:b0 + BC],
        ])
        idx16, perm16, sg_meta = _prep_core_padded(all_rows, max_counts)
        sg_meta_u = sg_meta
        im = {"feats": feats16, "idx16": idx16, "perm16": perm16}
        im.update(weights)
        im.update({k: np.ascontiguousarray(v) for k, v in biases.items()})
        in_maps2.append(im)

    nc = _build(float(b2.reshape(-1)[0]), sg_meta_u)
    trace = bool(int(os.environ.get("KBENCH_TRACE", "0")))
    res = run_bass_kernel_spmd(nc, in_maps2, core_ids=list(range(NCORES)), trace=trace)
    global LAST_EXEC_NS
    LAST_EXEC_NS = res.exec_time_ns
    outp = np.empty((B, 1), dtype=np.float32)
    for c in range(NCORES):
        outp[c * BC:(c + 1) * BC, 0] = res.results[c]["out"][0]
    return outp


def _prep_core_padded(all_rows, max_counts):
    """Like _prep_core but with per-(sg,bucket) padded counts fixed across
    cores so a single compiled kernel serves all cores."""
    sg_bounds = []
    a = 0
    for ngr in SG_GROUPS:
        sg_bounds.append((a, a + ngr * GROUP))
        a += ngr * GROUP
    idx_cols = []
    perm_cols = []
    sg_meta = []
    for sgi, (r0, r1) in enumerate(sg_bounds):
        rows = all_rows[r0:r1]
        bkt = rows // BKT
        loc = (rows % BKT).astype(np.int16)
        order = np.argsort(bkt, kind="stable")
        counts = np.bincount(bkt, minlength=NBKT)
        pcounts = np.asarray(max_counts[sgi], dtype=np.int64)
        assert (pcounts >= counts).all()
        starts = np.zeros(NBKT, dtype=np.int64)
        starts[1:] = np.cumsum(pcounts)[:-1]
        t_sg = int(pcounts.sum())
        lblk = t_sg // 128
        csort = np.cumsum(counts)
        before = np.concatenate([[0], csort[:-1]])
        ranks = np.empty(len(rows), dtype=np.int64)
        ranks[order] = np.arange(len(rows))
        pos = starts[bkt] + (ranks - before[bkt])
        idxv = np.zeros(t_sg, dtype=np.int16)
        srt = order
        bsrt = bkt[srt]
        within = np.arange(len(rows)) - before[bsrt]
        idxv[starts[bsrt] + within] = loc[srt]
        idx_cols.append(_wrap16(idxv))
        slots = (pos % 128) * lblk + pos // 128
        assert slots.max() < 32768, slots.max()
        for g in range(len(rows) // GROUP):
            perm_cols.append(_wrap16(slots[g * GROUP:(g + 1) * GROUP].astype(np.int16)))
        sg_meta.append({"t": t_sg, "starts": [int(x) for x in starts],
                        "counts": [int(x) for x in pcounts]})
    return (np.concatenate(idx_cols, axis=1),
            np.concatenate(perm_cols, axis=1), sg_meta)
